# revision 12
# baseline (speedup 1.0000x reference)
"""Trainium2 Bass kernel for nn_KnnGraph (topk_masking).

out = affinity * rowtop31mask * coltop31mask, zero diagonal.

Strategy (8 NeuronCores, SPMD):
- Row-shard: core c owns rows [c*1024, (c+1)*1024). It receives its row slice
  (xr) and, for the column side, the pre-transposed column slice (xcT) so both
  top-k reductions run along the free axis.
- Thresholds via exact candidate selection: per 128-row tile, 32 segment top-8s
  (contiguous 256-wide segments for rows; stride-32 "comb" segments for
  columns, which decorrelates consecutive-index clustering), then a
  4x max8 + 3x match_replace ladder over the 256 candidates gives the
  31st/32nd-largest values exactly.
- Mask pass: out = x * (max(Trow[p], Tcol[j]) <= x), with Tcol shared via a
  single packed AllGather (thresholds + kill indices in one collective).
- Boundary ties (31st == 32nd) are fixed exactly by scatter-writing 0.0 to the
  rank-32 cell of every row and every column (always a no-op unless there was
  a tie, in which case it kills exactly the over-kept duplicate; ties of
  multiplicity 2 are resolved identically to jax.lax.top_k's lowest-index
  preference via max_index's first-occurrence semantics). The diagonal is
  zeroed through the same scatter path. All kills are batched into two
  multi-offset indirect DMAs.
- Engine split: threshold search + kill max_index on DVE; mask compare
  (scalar_tensor_tensor) on GpSimd; final multiply split DVE/GpSimd to
  balance engine occupancy.
"""

import os
import sys
from contextlib import ExitStack

import numpy as np

for _p in ("/opt/trn_rl_repo", "/root/.axon_site/_ro/trn_rl_repo"):
    if os.path.isdir(_p) and _p not in sys.path:
        sys.path.append(_p)

import concourse.bass as bass
import concourse.tile as tile
from concourse import bacc, mybir
from concourse.bass import IndirectOffsetOnAxis
from concourse.bass_utils import run_bass_kernel_spmd

P = 128
NEGV = -3.0e38
F32 = mybir.dt.float32
U32 = mybir.dt.uint32


def build_nc(N=8192, C=8, enable_asserts=False, x_bufs=3, iters=1,
             multi_scatter=False, mult_dve_tiles=0):
    """Build the SPMD program (identical for all cores).

    iters > 1 wraps the computation in a hardware loop (For_i) for
    wall-clock differencing benchmarks; outputs are identical for any iters.
    multi_scatter: batch kill scatters into 2 multi-offset indirect DMAs
      (False: one indirect DMA per offset column, as the baseline did).
    mult_dve_tiles: how many of the 8 R-tiles run the final multiply on DVE
      (the rest run it on GpSimd), to balance engine time.
    """
    R = N // C            # rows (and cols) per core
    T = R // P            # 128-row tiles per core
    NSEG = 32             # segments per tile row
    NCAND = NSEG * 8
    KF = N // P           # free width when [N] is laid out as [P, KF]

    nc = bacc.Bacc(
        "TRN2",
        target_bir_lowering=False,
        debug=False,
        enable_asserts=enable_asserts,
        num_devices=C,
    )

    xr = nc.dram_tensor("xr", [R, N], F32, kind="ExternalInput")
    xcT = nc.dram_tensor("xcT", [R, N], F32, kind="ExternalInput")
    # per-core / static constants (f32; all integer-valued and < 2^24, exact)
    prowflat = nc.dram_tensor("prowflat", [P, 1], F32, kind="ExternalInput")  # p*N
    pbasef = nc.dram_tensor("pbasef", [P, 1], F32, kind="ExternalInput")      # c*R+p
    basef = nc.dram_tensor("basef", [P, 1], F32, kind="ExternalInput")        # c*R
    kiota = nc.dram_tensor("kiota", [P, KF], F32, kind="ExternalInput")       # k
    dumpdiag = nc.dram_tensor("dumpdiag", [P, KF], F32, kind="ExternalInput")
    out_t = nc.dram_tensor("out", [R, N], F32, kind="ExternalOutput")
    out_flat = out_t.ap().rearrange("a b -> (a b)")[:, None]  # [R*N, 1]

    with tile.TileContext(nc) as tc, ExitStack() as ctx:
        xpool = ctx.enter_context(tc.tile_pool(name="x", bufs=x_bufs))
        mpool = ctx.enter_context(tc.tile_pool(name="mask", bufs=2))
        cpool = ctx.enter_context(tc.tile_pool(name="cand", bufs=2))
        spool = ctx.enter_context(tc.tile_pool(name="small", bufs=4))
        stat = ctx.enter_context(tc.tile_pool(name="stat", bufs=1))
        dram = ctx.enter_context(tc.tile_pool(name="dram", bufs=1, space="DRAM"))

        # NB: collectives are invalid inside control flow, so benchmark
        # repetition (iters>1) must be unrolled, not a hardware loop.
        for _it in range(iters):
            # persistent state
            trow = stat.tile([P, T], F32, tag="trow")
            tcown = stat.tile([P, T], F32, tag="tcown")
            ckillf = stat.tile([P, T], F32, tag="ckillf")
            rdkill = stat.tile([P, 2 * T], U32, tag="rdkill")
            tcbc = stat.tile([P, N], F32, tag="tcbc")
            kioT = stat.tile([P, KF], F32, tag="kioT")
            ddT = stat.tile([P, KF], F32, tag="ddT")
            prT = stat.tile([P, 1], F32, tag="prT")
            pbT = stat.tile([P, 1], F32, tag="pbT")
            bfT = stat.tile([P, 1], F32, tag="bfT")
            zsW = max(2 * T, KF)
            zs = stat.tile([P, zsW], F32, tag="zs")

            nc.sync.dma_start(kioT[:], kiota.ap())
            nc.sync.dma_start(ddT[:], dumpdiag.ap())
            nc.sync.dma_start(prT[:], prowflat.ap())
            nc.sync.dma_start(pbT[:], pbasef.ap())
            nc.sync.dma_start(bfT[:], basef.ap())
            nc.gpsimd.memset(zs[:], 0.0)

            def thresholds(x, comb):
                """x: [P, N] tile -> (m3 [P,8] ranks 25-32 desc, idx [P,8] u32)."""
                cand = cpool.tile([P, NCAND], F32, tag="cand")
                if comb:
                    xv = x[:].rearrange("p (t s) -> p s t", s=NSEG)
                else:
                    xv = x[:].rearrange("p (s w) -> p s w", s=NSEG)
                for s in range(NSEG):
                    nc.vector.max(cand[:, s * 8:(s + 1) * 8], xv[:, s, :])
                for _ in range(3):
                    m = spool.tile([P, 8], F32, tag="mr")
                    nc.vector.max(m[:], cand[:])
                    nc.vector.match_replace(cand[:], m[:], cand[:], NEGV)
                m3 = spool.tile([P, 8], F32, tag="m3")
                nc.vector.max(m3[:], cand[:])
                idx = spool.tile([P, 8], U32, tag="idx")
                nc.vector.max_index(idx[:], m3[:], x[:])
                return m3, idx

            # ---- phase C: column thresholds + column kill rows ----
            for q in range(T):
                x = xpool.tile([P, N], F32, tag="x")
                nc.sync.dma_start(x[:], xcT.ap()[q * P:(q + 1) * P, :])
                m3, idx = thresholds(x, comb=True)
                nc.vector.tensor_copy(tcown[:, q:q + 1], m3[:, 6:7])
                # kill index as exact f32 (indices < 2^24)
                nc.vector.tensor_copy(ckillf[:, q:q + 1], idx[:, 7:8])

            # pack thresholds + kill indices into one collective:
            # pk_in[0, r] = tcol(r), pk_in[1, r] = colkill_row(r)
            pk_in = dram.tile([2, R], F32, tag="pk_in")
            pk_all = dram.tile([C, 2, R], F32, tag="pk_all")
            # bounce[q*P + p] = tile[p, q]
            nc.sync.dma_start(pk_in[0, :].rearrange("(q p) -> p q", p=P), tcown[:])
            nc.sync.dma_start(pk_in[1, :].rearrange("(q p) -> p q", p=P), ckillf[:])
            groups = [list(range(C))]
            nc.gpsimd.collective_compute(
                "AllGather", mybir.AluOpType.bypass, groups,
                ins=[pk_in[:].rearrange("a b -> (a b)").opt()],
                outs=[pk_all[:].rearrange("a b c -> (a b c)").opt()],
            )

            # broadcast Tcol over partitions
            nc.sync.dma_start(
                tcbc[:].rearrange("p (c r) -> p c r", c=C),
                pk_all[:, 0, :][None, :, :].to_broadcast([P, C, R]),
            )

            # decode column kills -> local flat offsets (f32 math, all exact)
            i2f = stat.tile([P, KF], F32, tag="i2f")
            nc.sync.dma_start(
                i2f[:], pk_all[:, 1, :].rearrange("c (b f) -> c b f", f=KF)
            )
            t1 = stat.tile([P, KF], F32, tag="t1")
            nc.vector.tensor_scalar(t1[:], i2f[:], bfT[:, 0:1], None,
                                    mybir.AluOpType.subtract)
            loc = stat.tile([P, KF], F32, tag="loc")
            nc.vector.scalar_tensor_tensor(loc[:], t1[:], float(N), kioT[:],
                                           mybir.AluOpType.mult, mybir.AluOpType.add)
            v1 = stat.tile([P, KF], F32, tag="v1")
            nc.vector.tensor_scalar(v1[:], t1[:], 0.0, None, mybir.AluOpType.is_ge)
            v2 = stat.tile([P, KF], F32, tag="v2")
            nc.vector.tensor_scalar(v2[:], t1[:], float(R), None, mybir.AluOpType.is_lt)
            nc.vector.tensor_tensor(v1[:], v1[:], v2[:], mybir.AluOpType.mult)
            # blend = (loc - dump)*valid + dump
            nc.vector.tensor_tensor(loc[:], loc[:], ddT[:], mybir.AluOpType.subtract)
            nc.vector.tensor_tensor(loc[:], loc[:], v1[:], mybir.AluOpType.mult)
            nc.vector.tensor_tensor(loc[:], loc[:], ddT[:], mybir.AluOpType.add)
            cko = stat.tile([P, KF], U32, tag="cko")
            nc.vector.tensor_copy(cko[:], loc[:])

            # ---- phase R (+3): row thresholds, kills, mask-multiply, write ----
            for t in range(T):
                x = xpool.tile([P, N], F32, tag="x")
                nc.sync.dma_start(x[:], xr.ap()[t * P:(t + 1) * P, :])
                m3, idx = thresholds(x, comb=False)
                nc.vector.tensor_copy(trow[:, t:t + 1], m3[:, 6:7])

                # row kill: (p*N + t*P*N) + j2 ; diag kill: p*N + t*P*N + (c*R+t*P+p)
                j2f = spool.tile([P, 1], F32, tag="j2f")
                nc.vector.tensor_copy(j2f[:], idx[:, 7:8])
                nc.vector.tensor_scalar(j2f[:], j2f[:], float(t * P * N), None,
                                        mybir.AluOpType.add)
                nc.vector.tensor_tensor(j2f[:], j2f[:], prT[:], mybir.AluOpType.add)
                nc.vector.tensor_copy(rdkill[:, t:t + 1], j2f[:])
                dkf = spool.tile([P, 1], F32, tag="dkf")
                nc.vector.tensor_scalar(dkf[:], pbT[:], float(t * P * N + t * P), None,
                                        mybir.AluOpType.add)
                nc.vector.tensor_tensor(dkf[:], dkf[:], prT[:], mybir.AluOpType.add)
                nc.vector.tensor_copy(rdkill[:, T + t:T + t + 1], dkf[:])

                mask = mpool.tile([P, N], F32, tag="mask")
                # TensorScalarPtr runs at 2x for f32 SBUF operands on DVE;
                # TensorTensor does not, so the multiply goes to GpSimd.
                nc.vector.scalar_tensor_tensor(mask[:], tcbc[:], trow[:, t:t + 1], x[:],
                                               mybir.AluOpType.max,
                                               mybir.AluOpType.is_le)
                mul_eng = nc.vector if t < mult_dve_tiles else nc.gpsimd
                mul_eng.tensor_tensor(mask[:], mask[:], x[:], mybir.AluOpType.mult)
                nc.sync.dma_start(out_t.ap()[t * P:(t + 1) * P, :], mask[:])
                # row + diag kill scatters for this tile, issued here so their
                # serial SWDGE emission overlaps later tiles' compute
                for k in (t, T + t):
                    nc.gpsimd.indirect_dma_start(
                        out=out_flat,
                        out_offset=IndirectOffsetOnAxis(ap=rdkill[:, k:k + 1], axis=0),
                        in_=zs[:, 0:1], in_offset=None,
                    )

            # ---- scatter kills (always-safe zero writes) ----
            # HW indirect DMA semantics: ONE offset per partition (offsets
            # [P,1]), writing in_'s data contiguously from that element —
            # multi-offset [P,K] calls scatter to wrong addresses on HW
            # (verified 2026-08: 112 bad cells), even though CoreSim accepts
            # them. So: one call per offset column.
            if multi_scatter:
                nc.gpsimd.indirect_dma_start(
                    out=out_flat,
                    out_offset=IndirectOffsetOnAxis(ap=rdkill[:, :], axis=0),
                    in_=zs[:, 0:2 * T], in_offset=None,
                )
                nc.gpsimd.indirect_dma_start(
                    out=out_flat,
                    out_offset=IndirectOffsetOnAxis(ap=cko[:, :], axis=0),
                    in_=zs[:, 0:KF], in_offset=None,
                )
            else:
                for k in range(KF):
                    nc.gpsimd.indirect_dma_start(
                        out=out_flat,
                        out_offset=IndirectOffsetOnAxis(ap=cko[:, k:k + 1], axis=0),
                        in_=zs[:, 0:1], in_offset=None,
                    )

    nc.compile()
    return nc


def make_in_maps(A, N=8192, C=8):
    R = N // C
    KF = N // P
    k = np.arange(N, dtype=np.float32).reshape(P, KF)
    q = (np.arange(N) % R).astype(np.float32).reshape(P, KF)
    in_maps = []
    for c in range(C):
        in_maps.append({
            "xr": np.ascontiguousarray(A[c * R:(c + 1) * R, :]),
            "xcT": np.ascontiguousarray(A[:, c * R:(c + 1) * R].T),
            "prowflat": (np.arange(P, dtype=np.float32) * N).reshape(P, 1),
            "pbasef": (c * R + np.arange(P, dtype=np.float32)).reshape(P, 1),
            "basef": np.full((P, 1), float(c * R), dtype=np.float32),
            "kiota": k,
            "dumpdiag": (q * N + c * R + q).astype(np.float32),
        })
    return in_maps


_NC_CACHE = {}


def kernel(affinity):
    A = np.ascontiguousarray(np.asarray(affinity, dtype=np.float32))
    N = A.shape[0]
    C = 8
    R = N // C
    if N not in _NC_CACHE:
        _NC_CACHE[N] = build_nc(N=N, C=C)
    nc = _NC_CACHE[N]
    in_maps = make_in_maps(A, N=N, C=C)
    res = run_bass_kernel_spmd(nc, in_maps, core_ids=list(range(C)))
    outs = res.results
    return np.concatenate([outs[c]["out"] for c in range(C)], axis=0)


if __name__ == "__main__":
    A = np.load("/tmp/A.npy")
    got = kernel(A)
    ref = np.load("/tmp/ref_out.npy")
    diff = (got != ref).sum()
    print("differing cells vs reference:", diff)


# revision 21
# speedup vs baseline: 2.8864x; 2.8864x over previous
"""Trainium2 Bass kernel for nn_KnnGraph (topk_masking).

out = affinity * rowtop31mask * coltop31mask, zero diagonal.

Strategy (8 NeuronCores, SPMD):
- Row-shard: core c owns rows [c*1024, (c+1)*1024). It receives its row slice
  (xr) and, for the column side, the pre-transposed column slice (xcT) so both
  top-k reductions run along the free axis.
- Thresholds via exact candidate selection: per 128-row tile, 32 segment top-8s
  (contiguous 256-wide segments for rows; stride-32 "comb" segments for
  columns), then a 4x max8 + 3x match_replace ladder over the 256 candidates
  gives the 31st/32nd-largest values exactly. max_index gives the rank-32
  cell's position (consume semantics resolve duplicate values to successive
  occurrences, matching jax.lax.top_k's lowest-index preference).
- One packed AllGather shares (Tcol, column-kill-row) across cores.
- Mask pass per row tile, all on fast TensorScalarPtr ops (2x f32 on DVE):
    m  = (max(Tcol[j], Trow[p]) <= x)        keep >= threshold
    m *= (killrow[j] != rowid[p])            column tie/rank-32 kill
    m *= (j != killcol[p])                   row tie/rank-32 kill
    m *= (j != rowid[p])                     diagonal zero
    out = m * x                              (GpSimd)
  The rank-32 kills are always-safe: that cell is below-threshold unless the
  31st and 32nd values tie, in which case exactly the over-kept duplicate
  dies. NO indirect scatter DMAs: scattered 4-byte HBM writes measured ~25us
  each (~2ms per pass), vs ~4.3us per fused full-tile compare.
"""

import os
import sys
from contextlib import ExitStack

import numpy as np

for _p in ("/opt/trn_rl_repo", "/root/.axon_site/_ro/trn_rl_repo"):
    if os.path.isdir(_p) and _p not in sys.path:
        sys.path.append(_p)

import concourse.bass as bass
import concourse.tile as tile
from concourse import bacc, mybir
from concourse.bass_utils import run_bass_kernel_spmd

P = 128
NEGV = -3.0e38
F32 = mybir.dt.float32
BF16 = mybir.dt.bfloat16
U32 = mybir.dt.uint32


def build_nc(N=8192, C=8, enable_asserts=False, x_bufs=2, iters=1,
             mult_dve_tiles=0, variant="full"):
    """Build the SPMD program (identical for all cores).

    iters > 1 unrolls the whole computation (for wall-clock differencing
    benchmarks; collectives are invalid inside control flow so no For_i).
    mult_dve_tiles: how many of the 8 R-tiles run the final multiply on DVE
      (the rest on GpSimd), to balance engine time.
    variant: ablation for benchmarking ("full", "no_maxindex", "no_mask",
      "no_cmax", "no_collective", "dma_only"). Only "full" is correct.
    """
    do_cthresh = variant not in ("dma_only", "no_cmax")
    do_rthresh = variant not in ("dma_only",)
    do_maxindex = variant not in ("no_maxindex", "dma_only")
    do_mask = variant not in ("no_mask", "dma_only")
    do_collective = variant not in ("no_collective", "dma_only")

    R = N // C            # rows (and cols) per core
    T = R // P            # 128-row tiles per core
    NSEG = 32             # segments per tile row
    NCAND = NSEG * 8

    nc = bacc.Bacc(
        "TRN2",
        target_bir_lowering=False,
        debug=False,
        enable_asserts=enable_asserts,
        num_devices=C,
    )

    xr = nc.dram_tensor("xr", [R, N], F32, kind="ExternalInput")
    xcT = nc.dram_tensor("xcT", [R, N], F32, kind="ExternalInput")
    # per-core / static constants (f32; all integer-valued and < 2^24, exact)
    pbasef = nc.dram_tensor("pbasef", [P, 1], F32, kind="ExternalInput")  # c*R+p
    iotan = nc.dram_tensor("iotan", [P, N], F32, kind="ExternalInput")    # j
    out_t = nc.dram_tensor("out", [R, N], F32, kind="ExternalOutput")

    with tile.TileContext(nc) as tc, ExitStack() as ctx:
        xpool = ctx.enter_context(tc.tile_pool(name="x", bufs=x_bufs))
        mpool = ctx.enter_context(tc.tile_pool(name="mask", bufs=2))
        cpool = ctx.enter_context(tc.tile_pool(name="cand", bufs=2))
        spool = ctx.enter_context(tc.tile_pool(name="small", bufs=4))
        stat = ctx.enter_context(tc.tile_pool(name="stat", bufs=1))
        dram = ctx.enter_context(tc.tile_pool(name="dram", bufs=1, space="DRAM"))

        # NB: collectives are invalid inside control flow, so benchmark
        # repetition (iters>1) must be unrolled, not a hardware loop.
        for _it in range(iters):
            # persistent state
            trow = stat.tile([P, T], F32, tag="trow")
            tcown = stat.tile([P, T], F32, tag="tcown")
            ckillf = stat.tile([P, T], F32, tag="ckillf")
            tcbc = stat.tile([P, N], F32, tag="tcbc")
            krbc = stat.tile([P, N], F32, tag="krbc")
            ioT = stat.tile([P, N], F32, tag="ioT")
            pbT = stat.tile([P, 1], F32, tag="pbT")

            nc.sync.dma_start(ioT[:], iotan.ap())
            nc.sync.dma_start(pbT[:], pbasef.ap())

            def thresholds(x, comb):
                """x: [P, N] tile -> (m3 [P,8] ranks 25-32 desc, idx [P,8] u32)."""
                cand = cpool.tile([P, NCAND], F32, tag="cand")
                if comb:
                    xv = x[:].rearrange("p (t s) -> p s t", s=NSEG)
                else:
                    xv = x[:].rearrange("p (s w) -> p s w", s=NSEG)
                for s in range(NSEG):
                    nc.vector.max(cand[:, s * 8:(s + 1) * 8], xv[:, s, :])
                for _ in range(3):
                    m = spool.tile([P, 8], F32, tag="mr")
                    nc.vector.max(m[:], cand[:])
                    nc.vector.match_replace(cand[:], m[:], cand[:], NEGV)
                m3 = spool.tile([P, 8], F32, tag="m3")
                nc.vector.max(m3[:], cand[:])
                idx = spool.tile([P, 8], U32, tag="idx")
                if do_maxindex:
                    nc.vector.max_index(idx[:], m3[:], x[:])
                else:
                    nc.vector.tensor_copy(idx[:], pbT[:, 0:1].to_broadcast([P, 8]))
                return m3, idx

            # ---- phase C: column thresholds + column kill rows ----
            for q in range(T):
                x = xpool.tile([P, N], F32, tag="x")
                nc.sync.dma_start(x[:], xcT.ap()[q * P:(q + 1) * P, :])
                if do_cthresh:
                    m3, idx = thresholds(x, comb=True)
                    nc.vector.tensor_copy(tcown[:, q:q + 1], m3[:, 6:7])
                    # kill row index as exact f32 (indices < 2^24)
                    nc.vector.tensor_copy(ckillf[:, q:q + 1], idx[:, 7:8])
                else:
                    nc.vector.memset(tcown[:, q:q + 1], 2.5)
                    nc.vector.memset(ckillf[:, q:q + 1], 0.0)

            # pack thresholds + kill rows into one collective:
            # pk_in[0, r] = tcol(r), pk_in[1, r] = colkill_row(r)
            pk_in = dram.tile([2, R], F32, tag="pk_in")
            pk_all = dram.tile([C, 2, R], F32, tag="pk_all")
            # bounce[q*P + p] = tile[p, q]
            nc.sync.dma_start(pk_in[0, :].rearrange("(q p) -> p q", p=P), tcown[:])
            nc.sync.dma_start(pk_in[1, :].rearrange("(q p) -> p q", p=P), ckillf[:])
            groups = [list(range(C))]
            if do_collective:
                nc.gpsimd.collective_compute(
                    "AllGather", mybir.AluOpType.bypass, groups,
                    ins=[pk_in[:].rearrange("a b -> (a b)").opt()],
                    outs=[pk_all[:].rearrange("a b c -> (a b c)").opt()],
                )
            else:
                for i in range(C):
                    nc.sync.dma_start(pk_all[i], pk_in[:])

            # broadcast Tcol and column-kill-row over partitions
            nc.sync.dma_start(
                tcbc[:].rearrange("p (c r) -> p c r", c=C),
                pk_all[:, 0, :][None, :, :].to_broadcast([P, C, R]),
            )
            nc.sync.dma_start(
                krbc[:].rearrange("p (c r) -> p c r", c=C),
                pk_all[:, 1, :][None, :, :].to_broadcast([P, C, R]),
            )

            # ---- phase R: row thresholds, fused mask+kills, multiply, write ----
            for t in range(T):
                x = xpool.tile([P, N], F32, tag="x")
                nc.sync.dma_start(x[:], xr.ap()[t * P:(t + 1) * P, :])
                if do_rthresh:
                    m3, idx = thresholds(x, comb=False)
                    nc.vector.tensor_copy(trow[:, t:t + 1], m3[:, 6:7])
                    # per-partition scalars for the kill compares
                    jkf = spool.tile([P, 1], F32, tag="jkf")   # row-kill column
                    nc.vector.tensor_copy(jkf[:], idx[:, 7:8])
                    rowid = spool.tile([P, 1], F32, tag="rowid")  # c*R + t*P + p
                    nc.vector.tensor_scalar(rowid[:], pbT[:], float(t * P), None,
                                            mybir.AluOpType.add)

                if do_mask:
                    mask = mpool.tile([P, N], BF16, tag="mask")
                    # all four passes are TensorScalarPtr: 2x f32 on DVE
                    nc.vector.scalar_tensor_tensor(mask[:], tcbc[:], trow[:, t:t + 1],
                                                   x[:], mybir.AluOpType.max,
                                                   mybir.AluOpType.is_le)
                    nc.vector.scalar_tensor_tensor(mask[:], krbc[:], rowid[:],
                                                   mask[:], mybir.AluOpType.not_equal,
                                                   mybir.AluOpType.mult)
                    nc.vector.scalar_tensor_tensor(mask[:], ioT[:], jkf[:],
                                                   mask[:], mybir.AluOpType.not_equal,
                                                   mybir.AluOpType.mult)
                    nc.vector.scalar_tensor_tensor(mask[:], ioT[:], rowid[:],
                                                   mask[:], mybir.AluOpType.not_equal,
                                                   mybir.AluOpType.mult)
                    # multiply in place into the x tile (x is dead afterwards)
                    mul_eng = nc.vector if t < mult_dve_tiles else nc.gpsimd
                    mul_eng.tensor_tensor(x[:], mask[:], x[:], mybir.AluOpType.mult)
                    nc.sync.dma_start(out_t.ap()[t * P:(t + 1) * P, :], x[:])
                else:
                    nc.sync.dma_start(out_t.ap()[t * P:(t + 1) * P, :], x[:])

    nc.compile()
    return nc


def make_in_maps(A, N=8192, C=8):
    R = N // C
    iotan = np.tile(np.arange(N, dtype=np.float32), (P, 1))
    AT = np.ascontiguousarray(A.T)
    in_maps = []
    for c in range(C):
        in_maps.append({
            "xr": A[c * R:(c + 1) * R, :],
            "xcT": AT[c * R:(c + 1) * R, :],
            "pbasef": (c * R + np.arange(P, dtype=np.float32)).reshape(P, 1),
            "iotan": iotan,
        })
    return in_maps


_NC_CACHE = {}


def kernel(affinity):
    A = np.ascontiguousarray(np.asarray(affinity, dtype=np.float32))
    N = A.shape[0]
    C = 8
    if N not in _NC_CACHE:
        _NC_CACHE[N] = build_nc(N=N, C=C)
    nc = _NC_CACHE[N]
    in_maps = make_in_maps(A, N=N, C=C)
    res = run_bass_kernel_spmd(nc, in_maps, core_ids=list(range(C)))
    outs = res.results
    return np.concatenate([outs[c]["out"] for c in range(C)], axis=0)


if __name__ == "__main__":
    A = np.load("/tmp/A.npy")
    got = kernel(A)
    ref = np.load("/tmp/ref_out.npy")
    diff = (got != ref).sum()
    print("differing cells vs reference:", diff)
